# revision 34
# baseline (speedup 1.0000x reference)
"""Trainium2 Bass kernel for MultiHeadedAttention (B=4,S=2048,D=1024,H=16).

Sharding: 8 cores = 4 batches x 2 head-groups (8 heads each). No
collectives: each core computes a partial output projection over its 512
attention channels; the host sums the two partials per batch and adds the
bias corrections (bo + Wo@bv).

v2 schedule: ScalarE (exp) is the pacing engine. The attention stream
runs 144 back-to-back ACTIVATE(exp) calls of [128,1024]; the PE stream is
organized so it never blocks ScalarE:
  - QK: scoresT = k_h @ q_h^T, two heads row-tiled (K=64) at
    tile_position (0,0)/(64,0) -> co-streamed, 512 cycles per pair.
  - PV: two heads col-tiled (M=64) at (0,0)/(0,64) into one PSUM bank,
    co-streamed, 512 cycles per pair.
  - Z (softmax denominator): separate col-tiled matmuls with a host-sent
    0/1 "kvones" stationary -> Z replicated to 64 partitions per head,
    aligned with the PV output for a direct elementwise normalize.
  - Projection chains (V/K/Q/out) fill the remaining PE slack via a
    token-bucket interleaver with emission deadlines.
Masking via KV compaction + zero-fill: padded K/V columns are zero, so
exp(0)=1 contributes v=0 to the numerator and kvones=0 to Z. No mask
bias needed. Normalize uses reciprocal_approx_fast (~51 ULP, fine at
rel-tol 2e-2).

PSUM budget (8 banks): sp ring bufs=3 x [128,1024]f32 (6 banks; scores
AND all projection chains share it) + pv bufs=1 (1) + zz bufs=1 (1).
"""

import sys

for _p in ("/opt/trn_rl_repo", "/root/.axon_site/_ro/trn_rl_repo"):
    if _p not in sys.path:
        sys.path.append(_p)

import numpy as np
import ml_dtypes

B, S, D, H = 4, 2048, 1024, 16
DK = D // H          # 64 head dim
NCORES = 8
HC = H // 2          # 8 heads per core
CH = HC * DK         # 512 channels per core
P = 128
NBLK = 512           # q block / moving free-dim block

bf16 = ml_dtypes.bfloat16


def _ceil_to(x, m):
    return ((x + m - 1) // m) * m


def build_nc(SKV, s=S, d=D, hc=HC):
    """Build the single-core Bass/Tile program (same program for all cores)."""
    import concourse.bass as bass
    import concourse.mybir as mybir
    import concourse.tile as tile

    dt = mybir.dt
    fp32 = dt.float32
    bft = dt.bfloat16
    Exp = mybir.ActivationFunctionType.Exp

    ch = hc * DK         # 512
    DC = d // P          # 8 contraction chunks for projections
    CT = ch // P         # 4 channel tiles (128 ch each = 2 heads = one "pr")
    L = SKV // P         # kv l-tiles
    NQ = s // NBLK       # query blocks
    MBLK = min(NBLK, d)
    MB = d // MBLK       # out-proj output blocks
    SCALE = 1.0 / np.sqrt(np.float32(DK))

    def kvblocks():
        out, b0 = [], 0
        while b0 < SKV:
            bs = min(NBLK, SKV - b0)
            out.append((b0, bs))
            b0 += bs
        return out

    KVB = kvblocks()

    nc = bass.Bass("TRN2", target_bir_lowering=False, debug=False)

    xqT = nc.dram_tensor("xqT", [d, s], bft, kind="ExternalInput").ap()
    xkT = nc.dram_tensor("xkT", [d, SKV], bft, kind="ExternalInput").ap()
    xvT = nc.dram_tensor("xvT", [d, SKV], bft, kind="ExternalInput").ap()
    wqT = nc.dram_tensor("wqT", [d, ch], bft, kind="ExternalInput").ap()
    wkT = nc.dram_tensor("wkT", [d, ch], bft, kind="ExternalInput").ap()
    wvT = nc.dram_tensor("wvT", [d, ch], bft, kind="ExternalInput").ap()
    woT = nc.dram_tensor("woT", [ch, d], bft, kind="ExternalInput").ap()
    bq2 = nc.dram_tensor("bq2", [P, CT], fp32, kind="ExternalInput").ap()
    bk2 = nc.dram_tensor("bk2", [P, CT], fp32, kind="ExternalInput").ap()
    kvo = nc.dram_tensor("kvo", [P, L * DK], bft, kind="ExternalInput").ap()
    out = nc.dram_tensor("out", [s, d], fp32, kind="ExternalOutput").ap()

    from contextlib import ExitStack

    with tile.TileContext(nc) as tc, ExitStack() as ctx:
        const = ctx.enter_context(tc.tile_pool(name="const", bufs=1))
        psc = ctx.enter_context(tc.tile_pool(name="psc", bufs=2, space="PSUM"))
        pproj = ctx.enter_context(tc.tile_pool(name="pproj", bufs=2,
                                               space="PSUM"))
        ppv = ctx.enter_context(tc.tile_pool(name="ppv", bufs=1, space="PSUM"))
        pzz = ctx.enter_context(tc.tile_pool(name="pzz", bufs=1, space="PSUM"))
        expp = ctx.enter_context(tc.tile_pool(name="expp", bufs=12))
        small = ctx.enter_context(tc.tile_pool(name="small", bufs=2))
        obuf = ctx.enter_context(tc.tile_pool(name="obuf", bufs=3))

        # ---- batched input DMAs, priority-ordered on two queues ----------
        # layout: x tensors as one tile [P, DC*len]; w tensors [P, DC*ch].
        wk_t = const.tile([P, DC * ch], bft, tag="wk", name="wk")
        xk_t = const.tile([P, DC * SKV], bft, tag="xk", name="xk")
        wq_t = const.tile([P, DC * ch], bft, tag="wq", name="wq")
        xq_t = const.tile([P, DC * s], bft, tag="xq", name="xq")
        wv_t = const.tile([P, DC * ch], bft, tag="wv", name="wv")
        xv_t = const.tile([P, DC * SKV], bft, tag="xv", name="xv")
        wo_t = const.tile([P, CT * d], bft, tag="wo", name="wo")
        bq_sb = const.tile([P, CT], fp32, tag="bq2", name="bq2")
        bk_sb = const.tile([P, CT], fp32, tag="bk2", name="bk2")
        kv_sb = const.tile([P, L * DK], bft, tag="kvo", name="kvo")

        def v3(t, n):   # [P, DC*n] tile -> [P, DC, n] view
            return t[:].rearrange("p (c n) -> p c n", n=n)

        # three trigger queues (~100 GB/s each observed), ordered so every
        # consumer's bytes land just before its emission deadline.
        def ld(eng, t, n, dram, c0, c1):
            eng.dma_start(out=v3(t, n)[:, :, c0:c1],
                          in_=dram[:, c0:c1].rearrange("(c p) x -> p c x", p=P))

        # NOTE: a DMA "trigger" occupies its issuing engine for the WHOLE
        # transfer. The scalar queue therefore carries ONLY the pre-stream
        # critical path (it finishes before the first ACTIVATE would run);
        # everything else rides sync/gpsimd.
        # scalar: pre-stream critical path only (engine-block is harmless
        # before the first ACTIVATE)
        ld(nc.scalar, wq_t, ch, wqT, 0, P)           # wq ct0 slice
        ld(nc.scalar, xq_t, s, xqT, 0, NBLK)
        # sync (slow, engine-executed): mid-priority items with late needs
        if L > 4:
            ld(nc.sync, xv_t, SKV, xvT, 4 * P, min(6 * P, SKV))
        if L > 6:
            ld(nc.sync, xv_t, SKV, xvT, 6 * P, SKV)
        for nq in range(1, NQ):
            ld(nc.sync, xq_t, s, xqT, nq * NBLK, (nq + 1) * NBLK)
        # gpsimd (fast HW DGE rings): the bulk, in deadline order
        nc.gpsimd.dma_start(out=kv_sb[:], in_=kvo[:, :])
        nc.gpsimd.dma_start(out=bq_sb[:], in_=bq2[:, :])
        nc.gpsimd.dma_start(out=bk_sb[:], in_=bk2[:, :])
        ld(nc.gpsimd, wk_t, ch, wkT, 0, P)           # wk ct0 slice
        ld(nc.gpsimd, xk_t, SKV, xkT, 0, min(NBLK, SKV))
        ld(nc.gpsimd, wv_t, ch, wvT, 0, ch)
        ld(nc.gpsimd, xv_t, SKV, xvT, 0, min(4 * P, SKV))
        ld(nc.gpsimd, wk_t, ch, wkT, P, 2 * P)       # wk ct1 slice
        if SKV > NBLK:
            ld(nc.gpsimd, xk_t, SKV, xkT, NBLK, min(2 * NBLK, SKV))
        if SKV > 2 * NBLK:
            ld(nc.gpsimd, xk_t, SKV, xkT, 2 * NBLK, SKV)
        ld(nc.gpsimd, wk_t, ch, wkT, 2 * P, ch)      # wk ct2/ct3
        ld(nc.gpsimd, wq_t, ch, wqT, P, 2 * P)       # wq ct1 slice
        ld(nc.gpsimd, wq_t, ch, wqT, 2 * P, ch)      # wq ct2/ct3
        ld(nc.gpsimd, wo_t, d, woT, 0, d)

        wk_v, wq_v, wv_v = v3(wk_t, ch), v3(wq_t, ch), v3(wv_t, ch)
        xk_v, xq_v, xv_v = v3(xk_t, SKV), v3(xq_t, s), v3(xv_t, SKV)
        wo_v = wo_t[:].rearrange("p (c n) -> p c n", n=d)

        # ---- persistent SBUF tiles --------------------------------------
        kT = [const.tile([P, SKV], bft, tag=f"kT{t}", name=f"kT{t}")
              for t in range(CT)]
        v_sb = [const.tile([P, ch], bft, tag=f"v{l}", name=f"v{l}")
                for l in range(L)]
        qTt = [[const.tile([P, NBLK], bft, tag=f"qT{t}_{q}", name=f"qT{t}_{q}")
                for q in range(NQ)] for t in range(CT)]
        att = [[const.tile([P, NBLK], bft, tag=f"at{t}_{q}", name=f"at{t}_{q}")
                for q in range(NQ)] for t in range(CT)]

        # ---- projection chain emitters (PE fillers) ----------------------
        def pp_tile():
            return pproj.tile([P, NBLK], fp32, tag="pp", name="pp")

        def vp_chain(l):
            ps = pp_tile()
            for dc in range(DC):
                nc.tensor.matmul(
                    ps[:, 0:ch], lhsT=xv_v[:, dc, l * P:(l + 1) * P],
                    rhs=wv_v[:, dc, :], start=(dc == 0), stop=(dc == DC - 1))
            nc.vector.tensor_copy(out=v_sb[l][:], in_=ps[:, 0:ch])

        def kp_chain(ct, bi):
            b0, bs = KVB[bi]
            ps = pp_tile()
            for dc in range(DC):
                nc.tensor.matmul(
                    ps[:, 0:bs], lhsT=wk_v[:, dc, ct * P:(ct + 1) * P],
                    rhs=xk_v[:, dc, b0:b0 + bs],
                    start=(dc == 0), stop=(dc == DC - 1))
            nc.vector.tensor_scalar_add(kT[ct][:, b0:b0 + bs], ps[:, 0:bs],
                                        bk_sb[:, ct:ct + 1])

        def qp_chain(nq, ct):
            q0 = nq * NBLK
            ps = pp_tile()
            for dc in range(DC):
                nc.tensor.matmul(
                    ps[:, 0:NBLK], lhsT=wq_v[:, dc, ct * P:(ct + 1) * P],
                    rhs=xq_v[:, dc, q0:q0 + NBLK],
                    start=(dc == 0), stop=(dc == DC - 1))
            nc.vector.tensor_scalar_add(qTt[ct][nq][:], ps[:, 0:NBLK],
                                        bq_sb[:, ct:ct + 1])

        def op_chain(nq, stl, mbi, alt=False):
            q0 = nq * NBLK + stl * P
            m0 = mbi * MBLK
            # in the tail the QK stream is done, so psc is free: ping-pong
            # between pproj and psc to break the 1-bank chain serialization
            ps = psc.tile([P, 2 * NBLK], fp32, tag="sp", name="sp") if alt \
                else pp_tile()
            for ct in range(CT):
                nc.tensor.matmul(
                    ps[:, 0:MBLK], lhsT=att[ct][nq][:, stl * P:(stl + 1) * P],
                    rhs=wo_v[:, ct, m0:m0 + MBLK],
                    start=(ct == 0), stop=(ct == CT - 1))
            ob = obuf.tile([P, MBLK], fp32, tag="ob", name="ob")
            nc.vector.tensor_copy(ob[:], ps[:, 0:MBLK])
            nc.sync.dma_start(out=out[q0:q0 + P, m0:m0 + MBLK], in_=ob[:])

        # ---- filler scheduler -------------------------------------------
        # (cost_cycles, deadline_slot_or_None, emit_fn)
        ZFIN_AT = 1        # prev pr: Z flush + zz copy + recip at this slot
        PVFIN_AT = 4       # prev pr: PV flush + normalize mul at this slot

        def slot_idx(nq, pr, l):
            return (nq * CT + pr) * L + l

        vp_emitted = set()

        def vp_chain_track(l):
            vp_chain(l)
            vp_emitted.add(l)

        queue = []
        for l in range(L):
            # PV(pr0, l) drains are gated on vp_emitted, so the deadline
            # only has to beat the forced flush at slot (pr1, PVFIN_AT).
            queue.append((8 * NBLK, min(slot_idx(0, 0, l) + 4,
                                        slot_idx(0, 1, 0) + PVFIN_AT - 1),
                          lambda l=l: vp_chain_track(l)))
        for bi in range(1, len(KVB)):
            # true need: QK(0,0,l=4*bi) reads kT[0] block bi
            queue.append((8 * KVB[bi][1], slot_idx(0, 0, min(4 * bi, L - 1)),
                          lambda bi=bi: kp_chain(0, bi)))
        for ct in range(1, CT):
            for bi in range(len(KVB)):
                dl = max(0, slot_idx(0, ct, min(4 * bi, L - 1)) - 2)
                queue.append((8 * KVB[bi][1], dl,
                              lambda ct=ct, bi=bi: kp_chain(ct, bi)))
            queue.append((8 * NBLK, max(0, slot_idx(0, ct, 0) - 2),
                          lambda ct=ct: qp_chain(0, ct)))
        for nq in range(1, NQ):
            for ct in range(CT):
                queue.append((8 * NBLK, slot_idx(nq, ct, 0) - 5,
                              lambda nq=nq, ct=ct: qp_chain(nq, ct)))
        # keep the queue deadline-sorted (None = +inf); out-proj chains are
        # inserted dynamically after each nq normalizes
        INF = 10 ** 9
        queue.sort(key=lambda c: c[1] if c[1] is not None else INF)

        def enqueue(cost, dl, fn):
            key = dl if dl is not None else INF
            i = len(queue)
            while i > 0 and (queue[i - 1][1] if queue[i - 1][1] is not None
                             else INF) > key:
                i -= 1
            queue.insert(i, (cost, dl, fn))

        budget = [0.0]

        def pop_fillers(cur_slot, force_deadlines):
            while queue:
                cost, dl, fn = queue[0]
                forced = force_deadlines and dl is not None and dl <= cur_slot
                if not forced and budget[0] < cost:
                    break
                queue.pop(0)
                fn()
                budget[0] -= cost

        # ---- attention stream -------------------------------------------
        SLOT_CYC = 1147 * 2.4          # PE cycles available per exp slot

        def qk(nq, pr, l):
            l0 = l * P
            sp = psc.tile([P, 2 * NBLK], fp32, tag="sp", name="sp")
            for hh in range(2):
                r0 = hh * DK
                nc.tensor.matmul(
                    sp[:, hh * NBLK:(hh + 1) * NBLK],
                    lhsT=kT[pr][r0:r0 + DK, l0:l0 + P],
                    rhs=qTt[pr][nq][r0:r0 + DK, :],
                    start=True, stop=True, tile_position=(r0, 0))
            e = expp.tile([P, 2 * NBLK], bft, tag="e", name="e")
            nc.scalar.activation(e[:], sp[:], Exp, scale=SCALE)
            return e

        # ---- prologue ----------------------------------------------------
        # PE warm-up: ~34 throwaway matmuls on the (tiny, early-loaded)
        # kvones tile keep the PE busy through the HAM window while the
        # first real inputs stream in, so the opening chains run at 2.4GHz.
        wu = pproj.tile([P, NBLK], fp32, tag="pp", name="wu")
        wuw = min(NBLK, L * DK)
        NWU = 24
        for i in range(NWU):
            # one long accumulation group: back-to-back issue, no
            # per-matmul PSUM drain serialization
            nc.tensor.matmul(wu[0:DK, 0:wuw], lhsT=kv_sb[:, 0:DK],
                             rhs=kv_sb[:, 0:wuw],
                             start=(i == 0), stop=(i == NWU - 1))
        kp_chain(0, 0)
        qp_chain(0, 0)

        # ---- main loop ---------------------------------------------------
        from collections import deque
        pvq: deque = deque()      # (seq, l, e, pv, slot)
        zq: deque = deque()       # (seq, l, e, zz, slot)
        zfin = [-1]               # highest seq whose finish_z was emitted
        pvfin = [-1]              # highest seq whose finish_pv was emitted

        def emit_pv(ent):
            seq, dl_, de, dpv, _ = ent
            dpr = (seq % CT)
            for hh in range(2):
                c0 = (2 * dpr + hh) * DK
                nc.tensor.matmul(
                    dpv[hh * DK:(hh + 1) * DK, :],
                    lhsT=v_sb[dl_][:, c0:c0 + DK],
                    rhs=de[:, hh * NBLK:(hh + 1) * NBLK],
                    start=(dl_ == 0), stop=(dl_ == L - 1),
                    tile_position=(0, hh * DK), skip_group_check=True)
            budget[0] -= NBLK

        def emit_z(ent):
            seq, dl_, de, dzz, _ = ent
            for hh in range(2):
                nc.tensor.matmul(
                    dzz[hh * DK:(hh + 1) * DK, :],
                    lhsT=kv_sb[:, dl_ * DK:(dl_ + 1) * DK],
                    rhs=de[:, hh * NBLK:(hh + 1) * NBLK],
                    start=(dl_ == 0), stop=(dl_ == L - 1),
                    tile_position=(0, hh * DK), skip_group_check=True)
            budget[0] -= NBLK

        def pv_ready(ent):
            # seq 0: its V tiles must exist; later seqs: the single pv bank
            # is free only once the previous pr's normalize mul is emitted
            if ent[0] == 0:
                return ent[1] in vp_emitted
            return ent[0] <= pvfin[0] + 1

        def z_ready(ent):
            return ent[0] <= zfin[0] + 1

        def drain_budgeted(cur):
            # budget shapes the drains, but never let the backlog exceed
            # ~3 slots (a long deque turns into a flush burst that starves
            # ScalarE at the pr boundary).
            while True:
                pv_ok = pvq and pvq[0][4] < cur and pv_ready(pvq[0])
                z_ok = zq and zq[0][4] < cur and z_ready(zq[0])
                pv_go = pv_ok and (len(pvq) > 3 or budget[0] >= NBLK)
                z_go = z_ok and (len(zq) > 3 or budget[0] >= NBLK)
                if pv_go and (not z_go or len(pvq) >= len(zq)):
                    emit_pv(pvq.popleft())
                elif z_go:
                    emit_z(zq.popleft())
                else:
                    break

        def finish_z(seq, zz):
            while zq and zq[0][0] == seq:
                emit_z(zq.popleft())
            zc = small.tile([P, NBLK], fp32, tag="zc", name="zc")
            nc.vector.tensor_copy(zc[:], zz[:])
            rz = small.tile([P, NBLK], fp32, tag="rz", name="rz")
            nc.vector.reciprocal(rz[:], zc[:])
            zfin[0] = seq
            return rz

        def finish_pv(seq, nq_, pr_, pv, rz):
            while pvq and pvq[0][0] == seq:
                emit_pv(pvq.popleft())
            nc.vector.tensor_mul(att[pr_][nq_][:], pv[:], rz[:])
            pvfin[0] = seq
            if pr_ == CT - 1:
                base = slot_idx(nq_ + 1, 0, 0) if nq_ + 1 < NQ else None
                tail = base is None
                for i, (stl, mbi) in enumerate(
                        (s_, m_) for s_ in range(NBLK // P)
                        for m_ in range(MB)):
                    dl = base + 4 + 4 * i if base is not None else None
                    enqueue(4 * MBLK, dl,
                            lambda n=nq_, s=stl, m=mbi, a=(tail and i % 2 == 1):
                            op_chain(n, s, m, a))

        prev = None               # (seq, nq, pr, pv, zz, rz)
        seq = -1
        for nq in range(NQ):
            for pr in range(CT):
                seq += 1
                pv = ppv.tile([P, NBLK], fp32, tag="pv", name="pv")
                zz = pzz.tile([P, NBLK], fp32, tag="zz", name="zz")
                for l in range(L):
                    cur = slot_idx(nq, pr, l)
                    budget[0] = max(budget[0] + SLOT_CYC - NBLK, -4096.0)
                    pop_fillers(cur, True)
                    e = qk(nq, pr, l)
                    if prev is not None:
                        if l == ZFIN_AT:
                            prev = prev[:5] + (finish_z(prev[0], prev[4]),)
                        elif l == PVFIN_AT:
                            finish_pv(prev[0], prev[1], prev[2], prev[3],
                                      prev[5])
                            prev = None
                    pvq.append((seq, l, e, pv, cur))
                    zq.append((seq, l, e, zz, cur))
                    drain_budgeted(cur)
                    pop_fillers(cur, False)
                prev = (seq, nq, pr, pv, zz, None)
        # ---- tail --------------------------------------------------------
        rz = finish_z(prev[0], prev[4])
        # keep the PE busy through the final reciprocal so HAM stays at
        # 8/8 and the closing out-proj chains run at full clock.
        dz = pzz.tile([P, NBLK], fp32, tag="zz", name="dz")
        for i in range(16):
            nc.tensor.matmul(dz[0:DK, 0:wuw], lhsT=kv_sb[:, 0:DK],
                             rhs=kv_sb[:, 0:wuw],
                             start=(i == 0), stop=(i == 15))
        finish_pv(prev[0], prev[1], prev[2], prev[3], rz)
        while queue:
            _, _, fn = queue.pop(0)
            fn()

    _split_mm_waits(nc)
    return nc


def _split_mm_waits(nc):
    """Walrus's compute-instruction encodings hold a single sync-wait
    command; Tile can emit instructions with 2+ waits ("Too many sync wait
    commands"). Move excess waits onto standalone EventSemaphore ops
    (which hold 2 waits each) inserted just before, on the same engine.
    Queue-based ops (DMA/Drain) tolerate multiple waits and are left."""
    import os
    import bass_rust
    import concourse.mybir as mybir

    limit = int(os.environ.get("SPLIT_LIMIT", "999999"))
    n = 0
    for f in nc.m.functions:
        for blk in f.blocks:
            out = []
            for inst in blk.instructions:
                si = inst.sync_info
                if si is not None and inst.opcode != "EventSemaphore":
                    cap = 1
                    waits = list(si.on_wait or [])
                    if len(waits) > cap and n < limit:
                        keep, extra = waits[-cap:], waits[:-cap]
                        while extra:
                            chunk, extra = extra[:2], extra[2:]
                            n += 1
                            out.append(mybir.InstEventSemaphore(
                                name=f"{inst.name}-evw{n}",
                                engine=inst.engine,
                                ins=[], outs=[],
                                sync_info=bass_rust.SyncInfo(
                                    on_wait=chunk, on_update=[]),
                            ))
                        inst.sync_info = bass_rust.SyncInfo(
                            on_wait=keep,
                            on_update=list(si.on_update or []))
                out.append(inst)
            blk.instructions = out
    return nc


def make_inmaps(query, key, value, mask, Wq, bq, Wk, bk, Wv, bv, Wo, bo):
    """Host-side shard/compact/transpose. Returns (in_maps, SKV)."""
    query = np.asarray(query, np.float32)
    key = np.asarray(key, np.float32)
    value = np.asarray(value, np.float32)
    mask = np.asarray(mask)
    Wq, Wk, Wv, Wo = (np.asarray(w, np.float32) for w in (Wq, Wk, Wv, Wo))
    bq, bk = np.asarray(bq, np.float32), np.asarray(bk, np.float32)

    idxs = []
    for b in range(B):
        idx = np.nonzero(np.asarray(mask[b, 0]) != 0)[0]
        if idx.size == 0:  # degenerate; unreachable for graded inputs
            idx = np.arange(S)
        idxs.append(idx)
    SKV = max(P, _ceil_to(max(len(i) for i in idxs), P))
    L = SKV // P
    CT = CH // P

    per_batch = []
    for b in range(B):
        idx = idxs[b]
        nv = len(idx)
        xk = np.zeros((SKV, D), np.float32)
        xk[:nv] = key[b][idx]
        xv = np.zeros((SKV, D), np.float32)
        xv[:nv] = value[b][idx]
        # kvones[p, l*DK + j] = 1.0 if kv slot l*128+p is valid else 0.0
        valid = (np.arange(SKV) < nv).astype(np.float32)       # [SKV]
        kvo = np.repeat(valid.reshape(L, P).T[:, :, None], DK, axis=2)
        per_batch.append(dict(
            xqT=np.ascontiguousarray(query[b].T).astype(bf16),
            xkT=np.ascontiguousarray(xk.T).astype(bf16),
            xvT=np.ascontiguousarray(xv.T).astype(bf16),
            kvo=np.ascontiguousarray(kvo.reshape(P, L * DK)).astype(bf16),
        ))

    in_maps = []
    for c in range(NCORES):
        b, g = divmod(c, 2)
        ch0 = g * CH
        m = dict(per_batch[b])
        m["wqT"] = np.ascontiguousarray(Wq[ch0:ch0 + CH].T).astype(bf16)
        m["wkT"] = np.ascontiguousarray(Wk[ch0:ch0 + CH].T).astype(bf16)
        m["wvT"] = np.ascontiguousarray(Wv[ch0:ch0 + CH].T).astype(bf16)
        m["woT"] = np.ascontiguousarray(Wo[:, ch0:ch0 + CH].T).astype(bf16)
        m["bq2"] = np.ascontiguousarray(bq[ch0:ch0 + CH].reshape(CT, P).T)
        m["bk2"] = np.ascontiguousarray(bk[ch0:ch0 + CH].reshape(CT, P).T)
        in_maps.append(m)
    return in_maps, SKV


def combine(results, Wo, bv, bo):
    Wo = np.asarray(Wo, np.float32)
    bv = np.asarray(bv, np.float32)
    bo = np.asarray(bo, np.float32)
    corr = (bo + Wo @ bv).astype(np.float32)
    final = np.empty((B, S, D), np.float32)
    for b in range(B):
        final[b] = results[2 * b]["out"] + results[2 * b + 1]["out"] + corr[None, :]
    return final


def kernel(query, key, value, mask, Wq, bq, Wk, bk, Wv, bv, Wo, bo):
    from concourse.bass_utils import run_bass_kernel_spmd

    in_maps, SKV = make_inmaps(query, key, value, mask,
                               Wq, bq, Wk, bk, Wv, bv, Wo, bo)
    nc = build_nc(SKV)
    res = run_bass_kernel_spmd(nc, in_maps, list(range(NCORES)))
    return combine(res.results, Wo, bv, bo)


if __name__ == "__main__":
    rng = np.random.default_rng(0)
    ins = dict(
        query=rng.standard_normal((B, S, D), np.float32),
        key=rng.standard_normal((B, S, D), np.float32),
        value=rng.standard_normal((B, S, D), np.float32),
        mask=(rng.integers(0, 2, (B, 1, S))).astype(np.int32),
        Wq=rng.standard_normal((D, D), np.float32) / 32,
        bq=np.zeros(D, np.float32),
        Wk=rng.standard_normal((D, D), np.float32) / 32,
        bk=np.zeros(D, np.float32),
        Wv=rng.standard_normal((D, D), np.float32) / 32,
        bv=np.zeros(D, np.float32),
        Wo=rng.standard_normal((D, D), np.float32) / 32,
        bo=np.zeros(D, np.float32),
    )
    out = kernel(**ins)
    print("out", out.shape, out.dtype, float(np.abs(out).mean()))


# revision 36
# speedup vs baseline: 1.0653x; 1.0653x over previous
"""Trainium2 Bass kernel for MultiHeadedAttention (B=4,S=2048,D=1024,H=16).

Sharding: 8 cores = 4 batches x 2 head-groups (8 heads each). No
collectives: each core computes a partial output projection over its 512
attention channels; the host sums the two partials per batch and adds the
bias corrections (bo + Wo@bv).

v2 schedule: ScalarE (exp) is the pacing engine. The attention stream
runs 144 back-to-back ACTIVATE(exp) calls of [128,1024]; the PE stream is
organized so it never blocks ScalarE:
  - QK: scoresT = k_h @ q_h^T, two heads row-tiled (K=64) at
    tile_position (0,0)/(64,0) -> co-streamed, 512 cycles per pair.
  - PV: two heads col-tiled (M=64) at (0,0)/(0,64) into one PSUM bank,
    co-streamed, 512 cycles per pair.
  - Z (softmax denominator): separate col-tiled matmuls with a host-sent
    0/1 "kvones" stationary -> Z replicated to 64 partitions per head,
    aligned with the PV output for a direct elementwise normalize.
  - Projection chains (V/K/Q/out) fill the remaining PE slack via a
    token-bucket interleaver with emission deadlines.
Masking via KV compaction + zero-fill: padded K/V columns are zero, so
exp(0)=1 contributes v=0 to the numerator and kvones=0 to Z. No mask
bias needed. Normalize uses reciprocal_approx_fast (~51 ULP, fine at
rel-tol 2e-2).

PSUM budget (8 banks): sp ring bufs=3 x [128,1024]f32 (6 banks; scores
AND all projection chains share it) + pv bufs=1 (1) + zz bufs=1 (1).
"""

import sys

for _p in ("/opt/trn_rl_repo", "/root/.axon_site/_ro/trn_rl_repo"):
    if _p not in sys.path:
        sys.path.append(_p)

import numpy as np
import ml_dtypes

B, S, D, H = 4, 2048, 1024, 16
DK = D // H          # 64 head dim
NCORES = 8
HC = H // 2          # 8 heads per core
CH = HC * DK         # 512 channels per core
P = 128
NBLK = 512           # q block / moving free-dim block

bf16 = ml_dtypes.bfloat16


def _ceil_to(x, m):
    return ((x + m - 1) // m) * m


def build_nc(SKV, s=S, d=D, hc=HC):
    """Build the single-core Bass/Tile program (same program for all cores)."""
    import concourse.bass as bass
    import concourse.mybir as mybir
    import concourse.tile as tile

    dt = mybir.dt
    fp32 = dt.float32
    bft = dt.bfloat16
    Exp = mybir.ActivationFunctionType.Exp

    ch = hc * DK         # 512
    DC = d // P          # 8 contraction chunks for projections
    CT = ch // P         # 4 channel tiles (128 ch each = 2 heads = one "pr")
    L = SKV // P         # kv l-tiles
    NQ = s // NBLK       # query blocks
    MBLK = min(NBLK, d)
    MB = d // MBLK       # out-proj output blocks
    SCALE = 1.0 / np.sqrt(np.float32(DK))

    def kvblocks():
        out, b0 = [], 0
        while b0 < SKV:
            bs = min(NBLK, SKV - b0)
            out.append((b0, bs))
            b0 += bs
        return out

    KVB = kvblocks()

    nc = bass.Bass("TRN2", target_bir_lowering=False, debug=False)

    xqT = nc.dram_tensor("xqT", [d, s], bft, kind="ExternalInput").ap()
    xkT = nc.dram_tensor("xkT", [d, SKV], bft, kind="ExternalInput").ap()
    xvT = nc.dram_tensor("xvT", [d, SKV], bft, kind="ExternalInput").ap()
    wqT = nc.dram_tensor("wqT", [d, ch], bft, kind="ExternalInput").ap()
    wkT = nc.dram_tensor("wkT", [d, ch], bft, kind="ExternalInput").ap()
    wvT = nc.dram_tensor("wvT", [d, ch], bft, kind="ExternalInput").ap()
    woT = nc.dram_tensor("woT", [ch, d], bft, kind="ExternalInput").ap()
    bq2 = nc.dram_tensor("bq2", [P, CT], fp32, kind="ExternalInput").ap()
    bk2 = nc.dram_tensor("bk2", [P, CT], fp32, kind="ExternalInput").ap()
    kvo = nc.dram_tensor("kvo", [P, L * DK], bft, kind="ExternalInput").ap()
    out = nc.dram_tensor("out", [s, d], fp32, kind="ExternalOutput").ap()

    from contextlib import ExitStack

    with tile.TileContext(nc) as tc, ExitStack() as ctx:
        const = ctx.enter_context(tc.tile_pool(name="const", bufs=1))
        psc = ctx.enter_context(tc.tile_pool(name="psc", bufs=2, space="PSUM"))
        pproj = ctx.enter_context(tc.tile_pool(name="pproj", bufs=2,
                                               space="PSUM"))
        ppv = ctx.enter_context(tc.tile_pool(name="ppv", bufs=1, space="PSUM"))
        pzz = ctx.enter_context(tc.tile_pool(name="pzz", bufs=1, space="PSUM"))
        expp = ctx.enter_context(tc.tile_pool(name="expp", bufs=13))
        small = ctx.enter_context(tc.tile_pool(name="small", bufs=2))
        obuf = ctx.enter_context(tc.tile_pool(name="obuf", bufs=3))

        # ---- batched input DMAs, priority-ordered on two queues ----------
        # layout: x tensors as one tile [P, DC*len]; w tensors [P, DC*ch].
        wk_t = const.tile([P, DC * ch], bft, tag="wk", name="wk")
        xk_t = const.tile([P, DC * SKV], bft, tag="xk", name="xk")
        wq_t = const.tile([P, DC * ch], bft, tag="wq", name="wq")
        xq_t = const.tile([P, DC * s], bft, tag="xq", name="xq")
        wv_t = const.tile([P, DC * ch], bft, tag="wv", name="wv")
        xv_t = const.tile([P, DC * SKV], bft, tag="xv", name="xv")
        wo_t = const.tile([P, CT * d], bft, tag="wo", name="wo")
        bq_sb = const.tile([P, CT], fp32, tag="bq2", name="bq2")
        bk_sb = const.tile([P, CT], fp32, tag="bk2", name="bk2")
        kv_sb = const.tile([P, L * DK], bft, tag="kvo", name="kvo")

        def v3(t, n):   # [P, DC*n] tile -> [P, DC, n] view
            return t[:].rearrange("p (c n) -> p c n", n=n)

        # three trigger queues (~100 GB/s each observed), ordered so every
        # consumer's bytes land just before its emission deadline.
        def ld(eng, t, n, dram, c0, c1):
            eng.dma_start(out=v3(t, n)[:, :, c0:c1],
                          in_=dram[:, c0:c1].rearrange("(c p) x -> p c x", p=P))

        # NOTE: a DMA "trigger" occupies its issuing engine for the WHOLE
        # transfer. The scalar queue therefore carries ONLY the pre-stream
        # critical path (it finishes before the first ACTIVATE would run);
        # everything else rides sync/gpsimd.
        # scalar: pre-stream critical path only (engine-block is harmless
        # before the first ACTIVATE)
        ld(nc.scalar, wq_t, ch, wqT, 0, P)           # wq ct0 slice
        ld(nc.scalar, xq_t, s, xqT, 0, NBLK)
        # sync (slow, engine-executed): mid-priority items with late needs
        if L > 4:
            ld(nc.sync, xv_t, SKV, xvT, 4 * P, min(6 * P, SKV))
        if L > 6:
            ld(nc.sync, xv_t, SKV, xvT, 6 * P, SKV)
        for nq in range(1, NQ):
            ld(nc.sync, xq_t, s, xqT, nq * NBLK, (nq + 1) * NBLK)
        # gpsimd (fast HW DGE rings): the bulk, in deadline order
        nc.gpsimd.dma_start(out=kv_sb[:], in_=kvo[:, :])
        nc.gpsimd.dma_start(out=bq_sb[:], in_=bq2[:, :])
        nc.gpsimd.dma_start(out=bk_sb[:], in_=bk2[:, :])
        ld(nc.gpsimd, wk_t, ch, wkT, 0, P)           # wk ct0 slice
        ld(nc.gpsimd, xk_t, SKV, xkT, 0, min(NBLK, SKV))
        ld(nc.gpsimd, wv_t, ch, wvT, 0, ch)
        ld(nc.gpsimd, xv_t, SKV, xvT, 0, min(4 * P, SKV))
        ld(nc.gpsimd, wk_t, ch, wkT, P, 2 * P)       # wk ct1 slice
        if SKV > NBLK:
            ld(nc.gpsimd, xk_t, SKV, xkT, NBLK, min(2 * NBLK, SKV))
        if SKV > 2 * NBLK:
            ld(nc.gpsimd, xk_t, SKV, xkT, 2 * NBLK, SKV)
        ld(nc.gpsimd, wk_t, ch, wkT, 2 * P, ch)      # wk ct2/ct3
        ld(nc.gpsimd, wq_t, ch, wqT, P, 2 * P)       # wq ct1 slice
        ld(nc.gpsimd, wq_t, ch, wqT, 2 * P, ch)      # wq ct2/ct3
        ld(nc.gpsimd, wo_t, d, woT, 0, d)

        wk_v, wq_v, wv_v = v3(wk_t, ch), v3(wq_t, ch), v3(wv_t, ch)
        xk_v, xq_v, xv_v = v3(xk_t, SKV), v3(xq_t, s), v3(xv_t, SKV)
        wo_v = wo_t[:].rearrange("p (c n) -> p c n", n=d)

        # ---- persistent SBUF tiles --------------------------------------
        kT = [const.tile([P, SKV], bft, tag=f"kT{t}", name=f"kT{t}")
              for t in range(CT)]
        v_sb = [const.tile([P, ch], bft, tag=f"v{l}", name=f"v{l}")
                for l in range(L)]
        qTt = [[const.tile([P, NBLK], bft, tag=f"qT{t}_{q}", name=f"qT{t}_{q}")
                for q in range(NQ)] for t in range(CT)]
        att = [[const.tile([P, NBLK], bft, tag=f"at{t}_{q}", name=f"at{t}_{q}")
                for q in range(NQ)] for t in range(CT)]

        # ---- projection chain emitters (PE fillers) ----------------------
        def pp_tile():
            return pproj.tile([P, NBLK], fp32, tag="pp", name="pp")

        def vp_chain(l):
            ps = pp_tile()
            for dc in range(DC):
                nc.tensor.matmul(
                    ps[:, 0:ch], lhsT=xv_v[:, dc, l * P:(l + 1) * P],
                    rhs=wv_v[:, dc, :], start=(dc == 0), stop=(dc == DC - 1))
            nc.vector.tensor_copy(out=v_sb[l][:], in_=ps[:, 0:ch])

        def kp_chain(ct, bi):
            b0, bs = KVB[bi]
            ps = pp_tile()
            for dc in range(DC):
                nc.tensor.matmul(
                    ps[:, 0:bs], lhsT=wk_v[:, dc, ct * P:(ct + 1) * P],
                    rhs=xk_v[:, dc, b0:b0 + bs],
                    start=(dc == 0), stop=(dc == DC - 1))
            nc.vector.tensor_scalar_add(kT[ct][:, b0:b0 + bs], ps[:, 0:bs],
                                        bk_sb[:, ct:ct + 1])

        def qp_chain(nq, ct):
            q0 = nq * NBLK
            ps = pp_tile()
            for dc in range(DC):
                nc.tensor.matmul(
                    ps[:, 0:NBLK], lhsT=wq_v[:, dc, ct * P:(ct + 1) * P],
                    rhs=xq_v[:, dc, q0:q0 + NBLK],
                    start=(dc == 0), stop=(dc == DC - 1))
            nc.vector.tensor_scalar_add(qTt[ct][nq][:], ps[:, 0:NBLK],
                                        bq_sb[:, ct:ct + 1])

        def op_chain(nq, stl, mbi, alt=False):
            q0 = nq * NBLK + stl * P
            m0 = mbi * MBLK
            # in the tail the QK stream is done, so psc is free: ping-pong
            # between pproj and psc to break the 1-bank chain serialization
            ps = psc.tile([P, 2 * NBLK], fp32, tag="sp", name="sp") if alt \
                else pp_tile()
            for ct in range(CT):
                nc.tensor.matmul(
                    ps[:, 0:MBLK], lhsT=att[ct][nq][:, stl * P:(stl + 1) * P],
                    rhs=wo_v[:, ct, m0:m0 + MBLK],
                    start=(ct == 0), stop=(ct == CT - 1))
            ob = obuf.tile([P, MBLK], fp32, tag="ob", name="ob")
            nc.vector.tensor_copy(ob[:], ps[:, 0:MBLK])
            nc.sync.dma_start(out=out[q0:q0 + P, m0:m0 + MBLK], in_=ob[:])

        # ---- filler scheduler -------------------------------------------
        # (cost_cycles, deadline_slot_or_None, emit_fn)
        FIN_AT = 2         # prev pr: flush + ring-freeing copies at this slot

        def slot_idx(nq, pr, l):
            return (nq * CT + pr) * L + l

        vp_emitted = set()

        def vp_chain_track(l):
            vp_chain(l)
            vp_emitted.add(l)

        queue = []
        for l in range(L):
            # PV(pr0, l) drains are gated on vp_emitted, so the deadline
            # only has to beat the forced flush at slot (pr1, PVFIN_AT).
            queue.append((8 * NBLK, min(slot_idx(0, 0, l) + 4,
                                        slot_idx(0, 1, 0) + FIN_AT - 1),
                          lambda l=l: vp_chain_track(l)))
        for bi in range(1, len(KVB)):
            # true need: QK(0,0,l=4*bi) reads kT[0] block bi
            queue.append((8 * KVB[bi][1], slot_idx(0, 0, min(4 * bi, L - 1)),
                          lambda bi=bi: kp_chain(0, bi)))
        for ct in range(1, CT):
            for bi in range(len(KVB)):
                dl = max(0, slot_idx(0, ct, min(4 * bi, L - 1)) - 2)
                queue.append((8 * KVB[bi][1], dl,
                              lambda ct=ct, bi=bi: kp_chain(ct, bi)))
            queue.append((8 * NBLK, max(0, slot_idx(0, ct, 0) - 2),
                          lambda ct=ct: qp_chain(0, ct)))
        for nq in range(1, NQ):
            for ct in range(CT):
                queue.append((8 * NBLK, slot_idx(nq, ct, 0) - 5,
                              lambda nq=nq, ct=ct: qp_chain(nq, ct)))
        # keep the queue deadline-sorted (None = +inf); out-proj chains are
        # inserted dynamically after each nq normalizes
        INF = 10 ** 9
        queue.sort(key=lambda c: c[1] if c[1] is not None else INF)

        def enqueue(cost, dl, fn):
            key = dl if dl is not None else INF
            i = len(queue)
            while i > 0 and (queue[i - 1][1] if queue[i - 1][1] is not None
                             else INF) > key:
                i -= 1
            queue.insert(i, (cost, dl, fn))

        budget = [0.0]

        def pop_fillers(cur_slot, force_deadlines):
            while queue:
                cost, dl, fn = queue[0]
                forced = force_deadlines and dl is not None and dl <= cur_slot
                if not forced and budget[0] < cost:
                    break
                queue.pop(0)
                fn()
                budget[0] -= cost

        # ---- attention stream -------------------------------------------
        SLOT_CYC = 1147 * 2.4          # PE cycles available per exp slot

        def qk(nq, pr, l):
            l0 = l * P
            sp = psc.tile([P, 2 * NBLK], fp32, tag="sp", name="sp")
            for hh in range(2):
                r0 = hh * DK
                nc.tensor.matmul(
                    sp[:, hh * NBLK:(hh + 1) * NBLK],
                    lhsT=kT[pr][r0:r0 + DK, l0:l0 + P],
                    rhs=qTt[pr][nq][r0:r0 + DK, :],
                    start=True, stop=True, tile_position=(r0, 0))
            e = expp.tile([P, 2 * NBLK], bft, tag="e", name="e")
            nc.scalar.activation(e[:], sp[:], Exp, scale=SCALE)
            return e

        # ---- prologue ----------------------------------------------------
        # PE warm-up: ~34 throwaway matmuls on the (tiny, early-loaded)
        # kvones tile keep the PE busy through the HAM window while the
        # first real inputs stream in, so the opening chains run at 2.4GHz.
        wu = pproj.tile([P, NBLK], fp32, tag="pp", name="wu")
        wuw = min(NBLK, L * DK)
        NWU = 24
        for i in range(NWU):
            # one long accumulation group: back-to-back issue, no
            # per-matmul PSUM drain serialization
            nc.tensor.matmul(wu[0:DK, 0:wuw], lhsT=kv_sb[:, 0:DK],
                             rhs=kv_sb[:, 0:wuw],
                             start=(i == 0), stop=(i == NWU - 1))
        kp_chain(0, 0)
        qp_chain(0, 0)

        # ---- main loop ---------------------------------------------------
        from collections import deque
        pvq: deque = deque()      # (seq, l, e, pv, slot)
        zq: deque = deque()       # (seq, l, e, zz, slot)
        zfin = [-1]               # highest seq whose finish_z was emitted
        pvfin = [-1]              # highest seq whose finish_pv was emitted

        def emit_pv(ent):
            seq, dl_, de, dpv, _ = ent
            dpr = (seq % CT)
            for hh in range(2):
                c0 = (2 * dpr + hh) * DK
                nc.tensor.matmul(
                    dpv[hh * DK:(hh + 1) * DK, :],
                    lhsT=v_sb[dl_][:, c0:c0 + DK],
                    rhs=de[:, hh * NBLK:(hh + 1) * NBLK],
                    start=(dl_ == 0), stop=(dl_ == L - 1),
                    tile_position=(0, hh * DK), skip_group_check=True)
            budget[0] -= NBLK

        def emit_z(ent):
            seq, dl_, de, dzz, _ = ent
            for hh in range(2):
                nc.tensor.matmul(
                    dzz[hh * DK:(hh + 1) * DK, :],
                    lhsT=kv_sb[:, dl_ * DK:(dl_ + 1) * DK],
                    rhs=de[:, hh * NBLK:(hh + 1) * NBLK],
                    start=(dl_ == 0), stop=(dl_ == L - 1),
                    tile_position=(0, hh * DK), skip_group_check=True)
            budget[0] -= NBLK

        def pv_ready(ent):
            # seq 0: its V tiles must exist; later seqs: the single pv bank
            # is free only once the previous pr's normalize mul is emitted
            if ent[0] == 0:
                return ent[1] in vp_emitted
            return ent[0] <= pvfin[0] + 1

        def z_ready(ent):
            return ent[0] <= zfin[0] + 1

        def drain_budgeted(cur):
            # budget shapes the drains, but never let the backlog exceed
            # ~3 slots (a long deque turns into a flush burst that starves
            # ScalarE at the pr boundary).
            while True:
                pv_ok = pvq and pvq[0][4] < cur and pv_ready(pvq[0])
                z_ok = zq and zq[0][4] < cur and z_ready(zq[0])
                pv_go = pv_ok and (len(pvq) > 3 or budget[0] >= NBLK)
                z_go = z_ok and (len(zq) > 3 or budget[0] >= NBLK)
                if pv_go and (not z_go or len(pvq) >= len(zq)):
                    emit_pv(pvq.popleft())
                elif z_go:
                    emit_z(zq.popleft())
                else:
                    break

        def finish_pr(seq, nq_, pr_, pv, zz):
            # drain leftovers, then free BOTH psum rings with fast copies
            # BEFORE the slow reciprocal enters the in-order DVE queue
            while zq and zq[0][0] == seq:
                emit_z(zq.popleft())
            while pvq and pvq[0][0] == seq:
                emit_pv(pvq.popleft())
            zc = small.tile([P, NBLK], fp32, tag="zc", name="zc")
            nc.vector.tensor_copy(zc[:], zz[:])
            pc = small.tile([P, NBLK], fp32, tag="pc", name="pc")
            nc.vector.tensor_copy(pc[:], pv[:])
            rz = small.tile([P, NBLK], fp32, tag="rz", name="rz")
            nc.vector.reciprocal(rz[:], zc[:])
            nc.vector.tensor_mul(att[pr_][nq_][:], pc[:], rz[:])
            zfin[0] = seq
            pvfin[0] = seq
            if pr_ == CT - 1:
                base = slot_idx(nq_ + 1, 0, 0) if nq_ + 1 < NQ else None
                tail = base is None
                for i, (stl, mbi) in enumerate(
                        (s_, m_) for s_ in range(NBLK // P)
                        for m_ in range(MB)):
                    dl = base + 4 + 4 * i if base is not None else None
                    enqueue(4 * MBLK, dl,
                            lambda n=nq_, s=stl, m=mbi, a=(tail and i % 2 == 1):
                            op_chain(n, s, m, a))

        prev = None               # (seq, nq, pr, pv, zz)
        seq = -1
        for nq in range(NQ):
            for pr in range(CT):
                seq += 1
                pv = ppv.tile([P, NBLK], fp32, tag="pv", name="pv")
                zz = pzz.tile([P, NBLK], fp32, tag="zz", name="zz")
                for l in range(L):
                    cur = slot_idx(nq, pr, l)
                    budget[0] = max(budget[0] + SLOT_CYC - NBLK, -4096.0)
                    pop_fillers(cur, True)
                    e = qk(nq, pr, l)
                    if prev is not None and l == FIN_AT:
                        finish_pr(prev[0], prev[1], prev[2], prev[3], prev[4])
                        prev = None
                    pvq.append((seq, l, e, pv, cur))
                    zq.append((seq, l, e, zz, cur))
                    drain_budgeted(cur)
                    pop_fillers(cur, False)
                prev = (seq, nq, pr, pv, zz)
        # ---- tail --------------------------------------------------------
        finish_pr(prev[0], prev[1], prev[2], prev[3], prev[4])
        # keep the PE busy through the final reciprocal so HAM stays at
        # 8/8 and the closing out-proj chains run at full clock.
        dz = pzz.tile([P, NBLK], fp32, tag="zz", name="dz")
        for i in range(16):
            nc.tensor.matmul(dz[0:DK, 0:wuw], lhsT=kv_sb[:, 0:DK],
                             rhs=kv_sb[:, 0:wuw],
                             start=(i == 0), stop=(i == 15))
        while queue:
            _, _, fn = queue.pop(0)
            fn()

    _split_mm_waits(nc)
    return nc


def _split_mm_waits(nc):
    """Walrus's compute-instruction encodings hold a single sync-wait
    command; Tile can emit instructions with 2+ waits ("Too many sync wait
    commands"). Move excess waits onto standalone EventSemaphore ops
    (which hold 2 waits each) inserted just before, on the same engine.
    Queue-based ops (DMA/Drain) tolerate multiple waits and are left."""
    import os
    import bass_rust
    import concourse.mybir as mybir

    limit = int(os.environ.get("SPLIT_LIMIT", "999999"))
    n = 0
    for f in nc.m.functions:
        for blk in f.blocks:
            out = []
            for inst in blk.instructions:
                si = inst.sync_info
                if si is not None and inst.opcode != "EventSemaphore":
                    cap = 1
                    waits = list(si.on_wait or [])
                    if len(waits) > cap and n < limit:
                        keep, extra = waits[-cap:], waits[:-cap]
                        while extra:
                            chunk, extra = extra[:2], extra[2:]
                            n += 1
                            out.append(mybir.InstEventSemaphore(
                                name=f"{inst.name}-evw{n}",
                                engine=inst.engine,
                                ins=[], outs=[],
                                sync_info=bass_rust.SyncInfo(
                                    on_wait=chunk, on_update=[]),
                            ))
                        inst.sync_info = bass_rust.SyncInfo(
                            on_wait=keep,
                            on_update=list(si.on_update or []))
                out.append(inst)
            blk.instructions = out
    return nc


def make_inmaps(query, key, value, mask, Wq, bq, Wk, bk, Wv, bv, Wo, bo):
    """Host-side shard/compact/transpose. Returns (in_maps, SKV)."""
    query = np.asarray(query, np.float32)
    key = np.asarray(key, np.float32)
    value = np.asarray(value, np.float32)
    mask = np.asarray(mask)
    Wq, Wk, Wv, Wo = (np.asarray(w, np.float32) for w in (Wq, Wk, Wv, Wo))
    bq, bk = np.asarray(bq, np.float32), np.asarray(bk, np.float32)

    idxs = []
    for b in range(B):
        idx = np.nonzero(np.asarray(mask[b, 0]) != 0)[0]
        if idx.size == 0:  # degenerate; unreachable for graded inputs
            idx = np.arange(S)
        idxs.append(idx)
    SKV = max(P, _ceil_to(max(len(i) for i in idxs), P))
    L = SKV // P
    CT = CH // P

    per_batch = []
    for b in range(B):
        idx = idxs[b]
        nv = len(idx)
        xk = np.zeros((SKV, D), np.float32)
        xk[:nv] = key[b][idx]
        xv = np.zeros((SKV, D), np.float32)
        xv[:nv] = value[b][idx]
        # kvones[p, l*DK + j] = 1.0 if kv slot l*128+p is valid else 0.0
        valid = (np.arange(SKV) < nv).astype(np.float32)       # [SKV]
        kvo = np.repeat(valid.reshape(L, P).T[:, :, None], DK, axis=2)
        per_batch.append(dict(
            xqT=np.ascontiguousarray(query[b].T).astype(bf16),
            xkT=np.ascontiguousarray(xk.T).astype(bf16),
            xvT=np.ascontiguousarray(xv.T).astype(bf16),
            kvo=np.ascontiguousarray(kvo.reshape(P, L * DK)).astype(bf16),
        ))

    in_maps = []
    for c in range(NCORES):
        b, g = divmod(c, 2)
        ch0 = g * CH
        m = dict(per_batch[b])
        m["wqT"] = np.ascontiguousarray(Wq[ch0:ch0 + CH].T).astype(bf16)
        m["wkT"] = np.ascontiguousarray(Wk[ch0:ch0 + CH].T).astype(bf16)
        m["wvT"] = np.ascontiguousarray(Wv[ch0:ch0 + CH].T).astype(bf16)
        m["woT"] = np.ascontiguousarray(Wo[:, ch0:ch0 + CH].T).astype(bf16)
        m["bq2"] = np.ascontiguousarray(bq[ch0:ch0 + CH].reshape(CT, P).T)
        m["bk2"] = np.ascontiguousarray(bk[ch0:ch0 + CH].reshape(CT, P).T)
        in_maps.append(m)
    return in_maps, SKV


def combine(results, Wo, bv, bo):
    Wo = np.asarray(Wo, np.float32)
    bv = np.asarray(bv, np.float32)
    bo = np.asarray(bo, np.float32)
    corr = (bo + Wo @ bv).astype(np.float32)
    final = np.empty((B, S, D), np.float32)
    for b in range(B):
        final[b] = results[2 * b]["out"] + results[2 * b + 1]["out"] + corr[None, :]
    return final


def kernel(query, key, value, mask, Wq, bq, Wk, bk, Wv, bv, Wo, bo):
    from concourse.bass_utils import run_bass_kernel_spmd

    in_maps, SKV = make_inmaps(query, key, value, mask,
                               Wq, bq, Wk, bk, Wv, bv, Wo, bo)
    nc = build_nc(SKV)
    res = run_bass_kernel_spmd(nc, in_maps, list(range(NCORES)))
    return combine(res.results, Wo, bv, bo)


if __name__ == "__main__":
    rng = np.random.default_rng(0)
    ins = dict(
        query=rng.standard_normal((B, S, D), np.float32),
        key=rng.standard_normal((B, S, D), np.float32),
        value=rng.standard_normal((B, S, D), np.float32),
        mask=(rng.integers(0, 2, (B, 1, S))).astype(np.int32),
        Wq=rng.standard_normal((D, D), np.float32) / 32,
        bq=np.zeros(D, np.float32),
        Wk=rng.standard_normal((D, D), np.float32) / 32,
        bk=np.zeros(D, np.float32),
        Wv=rng.standard_normal((D, D), np.float32) / 32,
        bv=np.zeros(D, np.float32),
        Wo=rng.standard_normal((D, D), np.float32) / 32,
        bo=np.zeros(D, np.float32),
    )
    out = kernel(**ins)
    print("out", out.shape, out.dtype, float(np.abs(out).mean()))
